# revision 3
# baseline (speedup 1.0000x reference)
"""AttnBlockST Trainium2 kernel — fp8 version.

Two SPMD phases on 8 NeuronCores:
  phase 1 (spatial): data-parallel over b*t (32 samples -> 4/core),
    attention over hw=1024 within each (bt, c, hw) sample.
  phase 2 (temporal): data-parallel over b*h*w (2048 -> 256/core),
    attention over t=16, 8 samples packed per 128-partition group with the
    block-diagonal logit mask folded into the S matmul as a rank-9 update.

Key optimizations vs the bf16 baseline:
  - All large matmuls run in fp8e4m3; contractions over 256 channels use
    DoubleRow perf mode (~1.8x PE throughput measured).
  - S = q^T k is folded algebraically: M = scale * Wq'^T Wk' is computed
    host-side, so queries are just h and only one projection (k' = M h)
    is needed.  Bias cross-terms in the logits shift softmax rows by
    ~5e-3 relative and are dropped (CPU-validated: 4.6e-3 total rel err).
  - Scale management keeps every fp8 tensor in its normal range:
    M is prescaled by ALPHA_K (descaled in the exp), Wv by ALPHA_V
    (descaled in the v^T copy), P-hat by S_PT (descaled in the O copy).
  - GN rstd computed as exp(-0.5*ln(var+eps)) so ACT stays on one
    table set (ln+exp share natural_log_exp_and_others; a Sqrt would
    force a ~2.7us table reload against the softmax Exps).
  - PSUM->SBUF copies spread across DVE and ACT (Identity with per-
    partition bias AP); SBUF-only elementwise offloaded to GpSimd.

GroupNorm affine (gamma/beta) is folded into the conv weights host-side.
Softmax and GN statistics are fp32/bf16.
"""

import numpy as np
import ml_dtypes
from contextlib import ExitStack

import concourse.bass as bass
import concourse.mybir as mybir
import concourse.tile as tile
from concourse.bass_utils import run_bass_kernel_spmd

# ---- walrus workaround: split multi-wait final drain ----
from concourse.vector_clock import ScopedClock
from concourse.tile import TileContext


def _patched_drain_and_barrier(self, tick_clock, wait_clock):
    nc = self.nc
    drain_inst = nc.sync.drain()
    wait_clock.add_sem_waits(
        drain_inst.ins, ScopedClock({None: tick_clock.global_clock})
    )
    si = drain_inst.ins.sync_info
    if si is not None and len(si.on_wait) > 1:
        waits = list(si.on_wait)
        drain_inst.ins.sync_info = mybir.SyncInfo(
            on_wait=waits[:1], on_update=list(si.on_update)
        )
        for w in waits[1:]:
            n = nc.sync.nop(nofuse=True, hint="drain_wait_split")
            n.ins.sync_info = mybir.SyncInfo(on_wait=[w], on_update=[])
    nc.all_engine_barrier()
    assert self.sems is not None
    popped = nc._tile_sem_poison_stack.pop()
    assert popped is self._sem_poison
    nc.clear_and_free_semaphores(list(self.sems.allocated().values()))
    nc.all_engine_barrier()


TileContext._drain_and_barrier = _patched_drain_and_barrier

# ---- problem constants (hardcoded per spec) ----
B, C, T, H, W = 2, 512, 16, 32, 32
GROUPS = 32
EPS = 1e-6
N_CORES = 8
P = 128
CCH = C // P          # 4 channel chunks
GPC = GROUPS // CCH   # 8 groups per 128-channel chunk
GS = C // GROUPS      # 16 channels per group

L1 = H * W            # 1024 spatial positions
NS1 = (B * T) // N_CORES   # 4 samples per core, phase 1
LCH1 = L1 // P        # 8 position chunks

NT2 = 16              # temporal length
NS2 = (B * H * W) // N_CORES  # 256 samples per core, phase 2
HALF = NS2 // 2       # process in halves of 128 samples
F2 = HALF * NT2       # 2048 free columns per half
NB2 = F2 // 512       # 4 n-blocks of 512
NGRP = F2 // P        # 16 groups of 8 samples per half

ALPHA_K = 64.0        # prescale on the folded M = s*Wq^T Wk (descaled in exp)
ALPHA_V = 16.0        # prescale on Wv (descaled in the v^T copy)
S_PT1 = 128.0         # P-hat scale, phase 1 (descaled in the O copy)
S_PT2 = 64.0          # P-hat scale, phase 2
MASK_A = 32.0         # block-mask rank-9 factors: A*B/ALPHA_K = 32 off-block
MASK_B = 64.0

F32 = mybir.dt.float32
BF16 = mybir.dt.bfloat16
F8 = mybir.dt.float8e4
AX = mybir.AxisListType.X
AF = mybir.ActivationFunctionType

NPF8 = ml_dtypes.float8_e4m3
NPBF = ml_dtypes.bfloat16


def _op():
    from concourse.alu_op_type import AluOpType
    return AluOpType


def _bcast_inner(ap, n):
    """View (P, F) access pattern as (P, F, n) with stride-0 inner dim."""
    return bass.AP(tensor=ap.tensor, offset=ap.offset, ap=list(ap.ap) + [[0, n]])


def _split_waits(nc, limit=1):
    """This walrus build rejects >1 sem wait on every ISA template tested
    (LDWEIGHTS, CTRL, ACT, DVE TensorScalar); hoist extra waits onto
    same-engine NoOps placed just before."""
    ctr = [0]
    for f in nc.m.functions:
        for b in f.blocks:
            new = []
            for ins in b.instructions:
                si = getattr(ins, "sync_info", None)
                waits = list(si.on_wait) if si is not None and si.on_wait else []
                lim = limit
                if len(waits) > lim:
                    for w in waits[lim:]:
                        ctr[0] += 1
                        new.append(mybir.InstNoOp(
                            name=f"wsplit-{ctr[0]}",
                            sync_info=mybir.SyncInfo(on_wait=[w], on_update=[]),
                            bass_nofuse=True,
                            engine=ins.engine,
                        ))
                    ins.sync_info = mybir.SyncInfo(
                        on_wait=waits[:lim], on_update=list(si.on_update)
                    )
                new.append(ins)
            b.instructions = new
    return nc


DR = mybir.MatmulPerfMode.DoubleRow


# ---------------------------------------------------------------- phase 1
def build_spatial(reps=1):
    nc = bass.Bass()
    xs = nc.dram_tensor("xs", [NS1, C, L1], F32, kind="ExternalInput")
    ys = nc.dram_tensor("ys", [NS1, C, L1], F32, kind="ExternalOutput")
    wd = {
        n: nc.dram_tensor(n, [C, C], F8, kind="ExternalInput")
        for n in ("wm", "wv", "wo")
    }
    bd = {
        n: nc.dram_tensor(n, [P, CCH], F32, kind="ExternalInput")
        for n in ("bv", "bo")
    }
    gmask_d = nc.dram_tensor("gmask", [P, GPC], BF16, kind="ExternalInput")
    bmask_d = nc.dram_tensor("bmask", [GPC, P], BF16, kind="ExternalInput")
    ident_d = nc.dram_tensor("ident", [P, P], F8, kind="ExternalInput")
    A = _op()

    with tile.TileContext(nc) as tc, ExitStack() as ctx:
        const = ctx.enter_context(tc.tile_pool(name="const", bufs=1))
        stp = ctx.enter_context(tc.tile_pool(name="stats", bufs=3))
        xp = ctx.enter_context(tc.tile_pool(name="x", bufs=2))
        hp = ctx.enter_context(tc.tile_pool(name="h", bufs=2))
        kp = ctx.enter_context(tc.tile_pool(name="k", bufs=2))
        vp = ctx.enter_context(tc.tile_pool(name="v", bufs=2))
        op_ = ctx.enter_context(tc.tile_pool(name="o", bufs=2))
        pp = ctx.enter_context(tc.tile_pool(name="pm", bufs=3))
        ptp = ctx.enter_context(tc.tile_pool(name="pt", bufs=2))
        yp = ctx.enter_context(tc.tile_pool(name="y", bufs=3))
        psA = ctx.enter_context(tc.tile_pool(name="psA", bufs=2, space="PSUM"))
        psB = ctx.enter_context(tc.tile_pool(name="psB", bufs=3, space="PSUM"))

        w_sb = {}
        for n in wd:
            t = const.tile([P, CCH, C], F8, tag=n)
            nc.sync.dma_start(out=t, in_=wd[n].rearrange("(k p) o -> p k o", p=P))
            w_sb[n] = t
        b_sb = {}
        for n in bd:
            t = const.tile([P, CCH], F32, tag=n)
            nc.sync.dma_start(out=t, in_=bd[n][:, :])
            b_sb[n] = t
        gmask = const.tile([P, GPC], BF16, tag="gmask")
        nc.sync.dma_start(out=gmask, in_=gmask_d[:, :])
        bmask = const.tile([GPC, P], BF16, tag="bmask")
        nc.sync.dma_start(out=bmask, in_=bmask_d[:, :])
        ident = const.tile([P, P], F8, tag="ident")
        nc.sync.dma_start(out=ident, in_=ident_d[:, :])
        eps_t = const.tile([GPC, 1], F32, tag="eps")
        nc.vector.memset(eps_t, EPS)

        def sample_body(i):
            x_sb = xp.tile([P, CCH, L1], F32)
            nc.sync.dma_start(out=x_sb, in_=xs[i].rearrange("(k p) l -> p k l", p=P))

            # ---- GroupNorm -> h (fp8) ----
            h_sb = hp.tile([P, CCH, L1], F8, tag="h")
            for k in range(CCH):
                xc = x_sb[:, k, :]
                st = stp.tile([P, 2, 6], F32, tag="bnst")
                nc.vector.bn_stats(out=st[:, 0, :], in_=xc[:, 0:512])
                nc.vector.bn_stats(out=st[:, 1, :], in_=xc[:, 512:1024])
                mv = stp.tile([P, 2], F32, tag="mv")
                nc.vector.bn_aggr(out=mv, in_=st)
                me = stp.tile([P, 2], BF16, tag="me")
                nc.vector.tensor_copy(out=me[:, 0:1], in_=mv[:, 0:1])
                m2 = stp.tile([P, 1], F32, tag="m2")
                nc.vector.tensor_mul(out=m2, in0=mv[:, 0:1], in1=mv[:, 0:1])
                nc.vector.tensor_add(out=me[:, 1:2], in0=mv[:, 1:2], in1=m2)
                gs_ps = psB.tile([GPC, 2], F32, tag="cv")
                nc.tensor.matmul(out=gs_ps, lhsT=gmask, rhs=me, start=True, stop=True)
                gs = stp.tile([GPC, 2], F32, tag="gs")
                nc.vector.tensor_copy(out=gs, in_=gs_ps)
                var = stp.tile([GPC, 1], F32, tag="var")
                nc.vector.tensor_mul(out=var, in0=gs[:, 0:1], in1=gs[:, 0:1])
                var2 = stp.tile([GPC, 1], F32, tag="var2")
                nc.vector.tensor_sub(out=var2, in0=gs[:, 1:2], in1=var)
                lnv = stp.tile([GPC, 1], F32, tag="lnv")
                nc.scalar.activation(out=lnv, in_=var2, func=AF.Ln, bias=eps_t)
                ab = stp.tile([GPC, 2], BF16, tag="ab")
                nc.scalar.activation(out=ab[:, 0:1], in_=lnv, func=AF.Exp,
                                     scale=-0.5)
                nc.vector.scalar_tensor_tensor(
                    out=ab[:, 1:2], in0=gs[:, 0:1], scalar=-1.0, in1=ab[:, 0:1],
                    op0=A.mult, op1=A.mult,
                )
                abc_ps = psB.tile([P, 2], F32, tag="cv")
                nc.tensor.matmul(out=abc_ps, lhsT=bmask, rhs=ab, start=True, stop=True)
                abc = stp.tile([P, 2], F32, tag="abc")
                nc.vector.tensor_copy(out=abc, in_=abc_ps)
                eng = nc.vector if k % 2 == 0 else nc.gpsimd
                eng.tensor_scalar(
                    out=h_sb[:, k, :], in0=xc,
                    scalar1=abc[:, 0:1], scalar2=abc[:, 1:2],
                    op0=A.mult, op1=A.add,
                )

            # ---- k' = (ALPHA_K * scale * Wq'^T Wk') h   (c-major layout) ----
            k_sb = kp.tile([P, CCH, L1], F8, tag="k")
            for m in range(CCH):
                for nb in range(2):
                    ps = psB.tile([P, 512], F32, tag="cv")
                    for pr in range(2):
                        nc.tensor.matmul(
                            out=ps,
                            lhsT=w_sb["wm"][:, 2 * pr:2 * pr + 2, m * P:(m + 1) * P],
                            rhs=h_sb[:, 2 * pr:2 * pr + 2, nb * 512:(nb + 1) * 512],
                            start=(pr == 0), stop=(pr == 1), perf_mode=DR,
                        )
                    dst = k_sb[:, m, nb * 512:(nb + 1) * 512]
                    if (m * 2 + nb) % 2 == 0:
                        nc.scalar.activation(out=dst, in_=ps, func=AF.Copy)
                    else:
                        nc.vector.tensor_copy(out=dst, in_=ps)

            # ---- v^T (positions on partitions), descale ALPHA_V ----
            vT_sb = vp.tile([P, LCH1, C], F8, tag="v")
            for m in range(LCH1):
                ps = psB.tile([P, C], F32, tag="cv")
                for pr in range(2):
                    nc.tensor.matmul(
                        out=ps,
                        lhsT=h_sb[:, 2 * pr:2 * pr + 2, m * P:(m + 1) * P],
                        rhs=w_sb["wv"][:, 2 * pr:2 * pr + 2, :],
                        start=(pr == 0), stop=(pr == 1), perf_mode=DR,
                    )
                dst = vT_sb[:, m, :]
                if m % 2 == 0:
                    nc.scalar.activation(out=dst, in_=ps, func=AF.Copy,
                                         scale=1.0 / ALPHA_V)
                else:
                    nc.vector.tensor_scalar_mul(out=dst, in0=ps,
                                                scalar1=1.0 / ALPHA_V)

            # ---- S = h^T k', softmax, P^T (normalized via diag trick) ----
            pt_sb = ptp.tile([P, LCH1, L1], F8, tag="ptv")
            for m in range(LCH1):
                ps_s = psA.tile([P, L1], F32, tag="mm")
                for nb in range(2):
                    for pr in range(2):
                        nc.tensor.matmul(
                            out=ps_s[:, nb * 512:(nb + 1) * 512],
                            lhsT=h_sb[:, 2 * pr:2 * pr + 2, m * P:(m + 1) * P],
                            rhs=k_sb[:, 2 * pr:2 * pr + 2, nb * 512:(nb + 1) * 512],
                            start=(pr == 0), stop=(pr == 1), perf_mode=DR,
                        )
                p_sb = pp.tile([P, L1], F8, tag="pv")
                rs = stp.tile([P, 1], F32, tag="rs")
                nc.scalar.activation(
                    out=p_sb, in_=ps_s, func=AF.Exp, scale=1.0 / ALPHA_K,
                    accum_out=rs,
                )
                rc = stp.tile([P, 1], F32, tag="rc")
                nc.vector.reciprocal(out=rc, in_=rs)
                dg = stp.tile([P, P], F8, tag="dg")
                nc.vector.tensor_scalar_mul(out=dg, in0=ident, scalar1=rc)
                for jq in range(2):
                    ps_t = psB.tile([P, 512], F32, tag="cv")
                    for j4 in range(4):
                        j = jq * 4 + j4
                        nc.tensor.matmul(
                            out=ps_t[:, j4 * P:(j4 + 1) * P],
                            lhsT=p_sb[:, j * P:(j + 1) * P], rhs=dg,
                            start=True, stop=True,
                        )
                    dst = pt_sb[:, jq * 4:(jq + 1) * 4, m * P:(m + 1) * P]
                    src = ps_t.rearrange("p (j q) -> p j q", q=P)
                    if jq == 0:
                        nc.vector.tensor_copy(out=dst, in_=src)
                    else:
                        nc.scalar.activation(out=dst, in_=src, func=AF.Copy)

            # ---- O = v P^T (c-major out), descale S_PT1, +bv ----
            o_sb = op_.tile([P, CCH, L1], F8, tag="o")
            for m in range(CCH):
                ps_o = psA.tile([P, L1], F32, tag="mm")
                for nb in range(2):
                    for jp in range(LCH1 // 2):
                        nc.tensor.matmul(
                            out=ps_o[:, nb * 512:(nb + 1) * 512],
                            lhsT=vT_sb[:, 2 * jp:2 * jp + 2, m * P:(m + 1) * P],
                            rhs=pt_sb[:, 2 * jp:2 * jp + 2, nb * 512:(nb + 1) * 512],
                            start=(jp == 0), stop=(jp == LCH1 // 2 - 1),
                            perf_mode=DR,
                        )
                dst = o_sb[:, m, :]
                if m % 2 == 0:
                    nc.vector.tensor_scalar(
                        out=dst, in0=ps_o,
                        scalar1=1.0 / S_PT1, scalar2=b_sb["bv"][:, m:m + 1],
                        op0=A.mult, op1=A.add,
                    )
                else:
                    nc.scalar.activation(
                        out=dst, in_=ps_o, func=AF.Identity,
                        scale=1.0 / S_PT1, bias=b_sb["bv"][:, m:m + 1],
                    )

            # ---- r = Wo O + bo + x -> ys ----
            for m in range(CCH):
                ps_r = psA.tile([P, L1], F32, tag="mm")
                for nb in range(2):
                    for pr in range(2):
                        nc.tensor.matmul(
                            out=ps_r[:, nb * 512:(nb + 1) * 512],
                            lhsT=w_sb["wo"][:, 2 * pr:2 * pr + 2, m * P:(m + 1) * P],
                            rhs=o_sb[:, 2 * pr:2 * pr + 2, nb * 512:(nb + 1) * 512],
                            start=(pr == 0), stop=(pr == 1), perf_mode=DR,
                        )
                y_sb = yp.tile([P, L1], F32, tag="y")
                if m % 2 == 0:
                    nc.vector.scalar_tensor_tensor(
                        out=y_sb, in0=ps_r, scalar=b_sb["bo"][:, m:m + 1],
                        in1=x_sb[:, m, :], op0=A.add, op1=A.add,
                    )
                else:
                    nc.scalar.activation(
                        out=y_sb, in_=ps_r, func=AF.Identity,
                        bias=b_sb["bo"][:, m:m + 1],
                    )
                    y2 = yp.tile([P, L1], F32, tag="y2")
                    nc.gpsimd.tensor_add(out=y2, in0=y_sb, in1=x_sb[:, m, :])
                    y_sb = y2
                nc.sync.dma_start(out=ys[i, m * P:(m + 1) * P, :], in_=y_sb)

        def reps_body(_iv=None):
            for i in range(NS1):
                sample_body(i)

        if reps > 1:
            with tc.For_i(0, reps, 1):
                reps_body()
        else:
            reps_body()
    return nc


# ---------------------------------------------------------------- phase 2
def build_temporal(reps=1):
    nc = bass.Bass()
    xt = nc.dram_tensor("xt", [C, NS2 * NT2], F32, kind="ExternalInput")
    yt = nc.dram_tensor("yt", [C, NS2 * NT2], F32, kind="ExternalOutput")
    wd = {
        n: nc.dram_tensor(n, [C, C], F8, kind="ExternalInput")
        for n in ("wm", "wv", "wo")
    }
    bd = {
        n: nc.dram_tensor(n, [P, CCH], F32, kind="ExternalInput")
        for n in ("bv", "bo")
    }
    gmask_d = nc.dram_tensor("gmask", [P, GPC], BF16, kind="ExternalInput")
    bmask_d = nc.dram_tensor("bmask", [GPC, P], BF16, kind="ExternalInput")
    ident_d = nc.dram_tensor("ident", [P, P], F8, kind="ExternalInput")
    mq_d = nc.dram_tensor("mq", [16, P], F8, kind="ExternalInput")
    mk_d = nc.dram_tensor("mk", [16, P], F8, kind="ExternalInput")
    A = _op()
    NN = HALF  # samples per half

    with tile.TileContext(nc) as tc, ExitStack() as ctx:
        const = ctx.enter_context(tc.tile_pool(name="const", bufs=1))
        stp = ctx.enter_context(tc.tile_pool(name="stats", bufs=3))
        xp = ctx.enter_context(tc.tile_pool(name="x", bufs=2))
        sqp = ctx.enter_context(tc.tile_pool(name="sq", bufs=2))
        tmpp = ctx.enter_context(tc.tile_pool(name="tmp", bufs=2))
        hp = ctx.enter_context(tc.tile_pool(name="h", bufs=1))
        kp = ctx.enter_context(tc.tile_pool(name="k", bufs=1))
        vp = ctx.enter_context(tc.tile_pool(name="v", bufs=1))
        op_ = ctx.enter_context(tc.tile_pool(name="o", bufs=1))
        pp = ctx.enter_context(tc.tile_pool(name="pm", bufs=3))
        yp = ctx.enter_context(tc.tile_pool(name="y", bufs=3))
        psA = ctx.enter_context(tc.tile_pool(name="psA", bufs=2, space="PSUM"))
        psC = ctx.enter_context(tc.tile_pool(name="psC", bufs=2, space="PSUM"))

        w_sb = {}
        for n in wd:
            t = const.tile([P, CCH, C], F8, tag=n)
            nc.sync.dma_start(out=t, in_=wd[n].rearrange("(k p) o -> p k o", p=P))
            w_sb[n] = t
        b_sb = {}
        for n in bd:
            t = const.tile([P, CCH], F32, tag=n)
            nc.sync.dma_start(out=t, in_=bd[n][:, :])
            b_sb[n] = t
        gmask = const.tile([P, GPC], BF16, tag="gmask")
        nc.sync.dma_start(out=gmask, in_=gmask_d[:, :])
        bmask = const.tile([GPC, P], BF16, tag="bmask")
        nc.sync.dma_start(out=bmask, in_=bmask_d[:, :])
        ident = const.tile([P, P], F8, tag="ident")
        nc.sync.dma_start(out=ident, in_=ident_d[:, :])
        mq = const.tile([16, P], F8, tag="mq")
        nc.sync.dma_start(out=mq, in_=mq_d[:, :])
        mk = const.tile([16, P], F8, tag="mk")
        nc.sync.dma_start(out=mk, in_=mk_d[:, :])
        eps_t = const.tile([GPC, 1], F32, tag="eps")
        nc.vector.memset(eps_t, EPS)

        xr = xt.rearrange("(k p) f -> p k f", p=P)
        yr = yt.rearrange("(k p) f -> p k f", p=P)

        def half_body(ih):
            f0 = ih * F2
            x_sb = xp.tile([P, CCH, F2], F32)
            nc.sync.dma_start(out=x_sb, in_=xr[:, :, f0:f0 + F2])

            # ---- GroupNorm over (16c x 16t) per sample ----
            h_sb = hp.tile([P, CCH, F2], F8, tag="h")
            for k in range(CCH):
                xc = x_sb[:, k, :]
                xc3 = x_sb[:, k, :].rearrange("p (n t) -> p n t", t=NT2)
                sq = sqp.tile([P, F2], BF16, tag="sq")
                nc.scalar.activation(out=sq, in_=xc, func=AF.Square)
                me = stp.tile([P, 2, NN], BF16, tag="me2")
                with nc.allow_low_precision(reason="GN stats tolerate bf16"):
                    nc.vector.reduce_sum(out=me[:, 0, :], in_=xc3, axis=AX)
                    nc.vector.reduce_sum(
                        out=me[:, 1, :],
                        in_=sq.rearrange("p (n t) -> p n t", t=NT2), axis=AX,
                    )
                gs_ps = psC.tile([GPC, 2, NN], F32, tag="pt")
                nc.tensor.matmul(
                    out=gs_ps.rearrange("g a n -> g (a n)"),
                    lhsT=gmask, rhs=me.rearrange("p a n -> p (a n)"),
                    start=True, stop=True,
                )
                gs = stp.tile([GPC, 2, NN], F32, tag="gs2")
                nc.vector.tensor_copy(out=gs, in_=gs_ps)
                var = stp.tile([GPC, NN], F32, tag="var2a")
                nc.vector.tensor_mul(out=var, in0=gs[:, 0, :], in1=gs[:, 0, :])
                var2 = stp.tile([GPC, NN], F32, tag="var2b")
                nc.vector.tensor_sub(out=var2, in0=gs[:, 1, :], in1=var)
                lnv = stp.tile([GPC, NN], F32, tag="lnv2")
                nc.scalar.activation(out=lnv, in_=var2, func=AF.Ln, bias=eps_t)
                ab = stp.tile([GPC, 2, NN], BF16, tag="ab2")
                nc.scalar.activation(out=ab[:, 0, :], in_=lnv, func=AF.Exp,
                                     scale=-0.5)
                nc.vector.scalar_tensor_tensor(
                    out=ab[:, 1, :], in0=gs[:, 0, :], scalar=-1.0, in1=ab[:, 0, :],
                    op0=A.mult, op1=A.mult,
                )
                abc_ps = psC.tile([P, 2, NN], F32, tag="pt")
                nc.tensor.matmul(
                    out=abc_ps.rearrange("p a n -> p (a n)"),
                    lhsT=bmask, rhs=ab.rearrange("g a n -> g (a n)"),
                    start=True, stop=True,
                )
                abc = stp.tile([P, 2, NN], F32, tag="abc2")
                nc.vector.tensor_copy(out=abc, in_=abc_ps)
                tmp = tmpp.tile([P, F2], BF16, tag="tmp")
                eng = nc.vector if k % 2 == 0 else nc.gpsimd
                eng.tensor_tensor(
                    out=tmp.rearrange("p (n t) -> p n t", t=NT2),
                    in0=xc3, in1=_bcast_inner(abc[:, 0, :], NT2), op=A.mult,
                )
                nc.gpsimd.tensor_tensor(
                    out=h_sb[:, k, :].rearrange("p (n t) -> p n t", t=NT2),
                    in0=tmp.rearrange("p (n t) -> p n t", t=NT2),
                    in1=_bcast_inner(abc[:, 1, :], NT2), op=A.add,
                )

            # ---- k' projection ----
            k_sb = kp.tile([P, CCH, F2], F8, tag="k")
            for m in range(CCH):
                for nb in range(NB2):
                    ps = psA.tile([P, 512], F32, tag="mm")
                    for pr in range(2):
                        nc.tensor.matmul(
                            out=ps,
                            lhsT=w_sb["wm"][:, 2 * pr:2 * pr + 2, m * P:(m + 1) * P],
                            rhs=h_sb[:, 2 * pr:2 * pr + 2, nb * 512:(nb + 1) * 512],
                            start=(pr == 0), stop=(pr == 1), perf_mode=DR,
                        )
                    dst = k_sb[:, m, nb * 512:(nb + 1) * 512]
                    if (m * NB2 + nb) % 2 == 0:
                        nc.scalar.activation(out=dst, in_=ps, func=AF.Copy)
                    else:
                        nc.vector.tensor_copy(out=dst, in_=ps)

            # ---- v^T ----
            vT_sb = vp.tile([P, NGRP, C], F8, tag="v")
            for m in range(NGRP):
                ps = psA.tile([P, C], F32, tag="mm")
                for pr in range(2):
                    nc.tensor.matmul(
                        out=ps,
                        lhsT=h_sb[:, 2 * pr:2 * pr + 2, m * P:(m + 1) * P],
                        rhs=w_sb["wv"][:, 2 * pr:2 * pr + 2, :],
                        start=(pr == 0), stop=(pr == 1), perf_mode=DR,
                    )
                dst = vT_sb[:, m, :]
                if m % 2 == 0:
                    nc.scalar.activation(out=dst, in_=ps, func=AF.Copy,
                                         scale=1.0 / ALPHA_V)
                else:
                    nc.vector.tensor_scalar_mul(out=dst, in0=ps,
                                                scalar1=1.0 / ALPHA_V)

            # ---- attention per 8-sample group (block mask via rank-9 S update) ----
            o_sb = op_.tile([P, CCH, F2], F8, tag="o")
            for g in range(NGRP):
                c0 = g * P
                ps_s = psC.tile([P, P], F32, tag="so", bufs=4)
                for kk in range(CCH):
                    nc.tensor.matmul(
                        out=ps_s,
                        lhsT=h_sb[:, kk, c0:c0 + P],
                        rhs=k_sb[:, kk, c0:c0 + P],
                        start=(kk == 0), stop=False,
                    )
                nc.tensor.matmul(
                    out=ps_s, lhsT=mq, rhs=mk, start=False, stop=True,
                )
                p_sb = pp.tile([P, P], F8, tag="pv")
                rs = stp.tile([P, 1], F32, tag="rs")
                nc.scalar.activation(
                    out=p_sb, in_=ps_s, func=AF.Exp, scale=1.0 / ALPHA_K,
                    accum_out=rs,
                )
                rc = stp.tile([P, 1], F32, tag="rc")
                nc.vector.reciprocal(out=rc, in_=rs)
                dg = stp.tile([P, P], F8, tag="dg")
                nc.vector.tensor_scalar_mul(out=dg, in0=ident, scalar1=rc)
                ps_t = psC.tile([P, P], F32, tag="so", bufs=4)
                nc.tensor.matmul(out=ps_t, lhsT=p_sb, rhs=dg, start=True, stop=True)
                pt_sb = pp.tile([P, P], F8, tag="ptv")
                if g % 2 == 0:
                    nc.vector.tensor_copy(out=pt_sb, in_=ps_t)
                else:
                    nc.scalar.activation(out=pt_sb, in_=ps_t, func=AF.Copy)
                for m in range(CCH):
                    ps_o = psC.tile([P, P], F32, tag="so", bufs=4)
                    nc.tensor.matmul(
                        out=ps_o, lhsT=vT_sb[:, g, m * P:(m + 1) * P], rhs=pt_sb,
                        start=True, stop=True,
                    )
                    dst = o_sb[:, m, c0:c0 + P]
                    if m % 2 == 0:
                        nc.vector.tensor_scalar(
                            out=dst, in0=ps_o,
                            scalar1=1.0 / S_PT2, scalar2=b_sb["bv"][:, m:m + 1],
                            op0=A.mult, op1=A.add,
                        )
                    else:
                        nc.scalar.activation(
                            out=dst, in_=ps_o, func=AF.Identity,
                            scale=1.0 / S_PT2, bias=b_sb["bv"][:, m:m + 1],
                        )

            # ---- r = Wo O + bo + x -> yt ----
            for m in range(CCH):
                for nb in range(NB2):
                    ps_r = psA.tile([P, 512], F32, tag="mm")
                    for pr in range(2):
                        nc.tensor.matmul(
                            out=ps_r,
                            lhsT=w_sb["wo"][:, 2 * pr:2 * pr + 2, m * P:(m + 1) * P],
                            rhs=o_sb[:, 2 * pr:2 * pr + 2, nb * 512:(nb + 1) * 512],
                            start=(pr == 0), stop=(pr == 1), perf_mode=DR,
                        )
                    y_sb = yp.tile([P, 512], F32, tag="y")
                    if (m * NB2 + nb) % 2 == 0:
                        nc.vector.scalar_tensor_tensor(
                            out=y_sb, in0=ps_r, scalar=b_sb["bo"][:, m:m + 1],
                            in1=x_sb[:, m, nb * 512:(nb + 1) * 512],
                            op0=A.add, op1=A.add,
                        )
                    else:
                        nc.scalar.activation(
                            out=y_sb, in_=ps_r, func=AF.Identity,
                            bias=b_sb["bo"][:, m:m + 1],
                        )
                        y2 = yp.tile([P, 512], F32, tag="y2")
                        nc.gpsimd.tensor_add(
                            out=y2, in0=y_sb,
                            in1=x_sb[:, m, nb * 512:(nb + 1) * 512],
                        )
                        y_sb = y2
                    nc.sync.dma_start(
                        out=yr[:, m, f0 + nb * 512:f0 + (nb + 1) * 512], in_=y_sb
                    )

        def reps_body(_iv=None):
            for ih in range(2):
                half_body(ih)

        if reps > 1:
            with tc.For_i(0, reps, 1):
                reps_body()
        else:
            reps_body()
    return nc


# ---------------------------------------------------------------- host side
def _q8(x):
    return np.clip(np.asarray(x, np.float32), -240, 240).astype(NPF8)


def _fold(inputs, sfx):
    """Host-side weight folds for one phase. Returns dict of device arrays."""
    g = np.asarray(inputs[f"gamma_{sfx}"], np.float32)
    be = np.asarray(inputs[f"beta_{sfx}"], np.float32)
    wq = np.asarray(inputs[f"wq_{sfx}"], np.float32) * g[None, :]
    wk = np.asarray(inputs[f"wk_{sfx}"], np.float32) * g[None, :]
    wv = np.asarray(inputs[f"wv_{sfx}"], np.float32) * g[None, :]
    wo = np.asarray(inputs[f"wo_{sfx}"], np.float32)
    bv = (np.asarray(inputs[f"bv_{sfx}"], np.float32)
          + np.asarray(inputs[f"wv_{sfx}"], np.float32) @ be)
    bo = np.asarray(inputs[f"bo_{sfx}"], np.float32)
    scale = float(C) ** -0.5
    M = ALPHA_K * scale * (wq.T @ wk)           # k' = M h, S = h^T k'
    MT = np.ascontiguousarray(M.T)              # lhsT layout: [c_contract, c_out]
    wvT = np.ascontiguousarray((ALPHA_V * wv).T)
    woT = np.ascontiguousarray(wo.T)
    return dict(
        wm=_q8(MT), wv=_q8(wvT), wo=_q8(woT),
        bv=np.ascontiguousarray(bv.reshape(CCH, P).T),
        bo=np.ascontiguousarray(bo.reshape(CCH, P).T),
    )


def _consts():
    gmask1 = np.zeros((P, GPC), np.float32)
    for p in range(P):
        gmask1[p, p // GS] = 1.0 / (GS * 1)  # spatial: /16 (channel avg of means)
    gmask2 = np.zeros((P, GPC), np.float32)
    for p in range(P):
        gmask2[p, p // GS] = 1.0 / (GS * NT2)  # temporal: /256 (full group sum)
    bmask = np.zeros((GPC, P), np.float32)
    for p in range(P):
        bmask[p // GS, p] = 1.0
    ident1 = (S_PT1 * np.eye(P)).astype(NPF8)
    ident2 = (S_PT2 * np.eye(P)).astype(NPF8)
    mq = np.zeros((16, P), np.float32)
    mk = np.zeros((16, P), np.float32)
    mq[0, :] = -MASK_A
    mk[0, :] = MASK_B
    for s in range(P // NT2):
        mq[1 + s, s * NT2:(s + 1) * NT2] = MASK_A
        mk[1 + s, s * NT2:(s + 1) * NT2] = MASK_B
    return (gmask1.astype(NPBF), gmask2.astype(NPBF), bmask.astype(NPBF),
            ident1, ident2, mq.astype(NPF8), mk.astype(NPF8))


_CACHE = {}


def kernel(**inputs):
    x = np.asarray(inputs["x"], np.float32)
    gmask1, gmask2, bmask, ident1, ident2, mq, mk = _consts()

    f1 = _fold(inputs, "s")
    f2 = _fold(inputs, "t")

    if "nc1" not in _CACHE:
        _CACHE["nc1"] = _split_waits(build_spatial())
        _CACHE["nc2"] = _split_waits(build_temporal())
    nc1, nc2 = _CACHE["nc1"], _CACHE["nc2"]

    # ---- phase 1: spatial over (b t) ----
    xs = np.ascontiguousarray(
        x.transpose(0, 2, 1, 3, 4).reshape(B * T, C, L1)
    )
    common1 = dict(gmask=gmask1, bmask=bmask, ident=ident1, **f1)
    in_maps1 = [
        dict(xs=np.ascontiguousarray(xs[i * NS1:(i + 1) * NS1]), **common1)
        for i in range(N_CORES)
    ]
    _CACHE["in_maps1"] = in_maps1
    r1 = run_bass_kernel_spmd(nc1, in_maps1, core_ids=list(range(N_CORES)))
    _CACHE["last_r1"] = [r1.results[i]["ys"] for i in range(N_CORES)]
    ys = np.concatenate([r1.results[i]["ys"] for i in range(N_CORES)], axis=0)

    # ---- phase 2: temporal over (b h w) ----
    x2 = ys.reshape(B, T, C, H, W).transpose(0, 3, 4, 2, 1)  # (b,h,w,c,t)
    x2 = x2.reshape(B * H * W, C, NT2)
    common2 = dict(gmask=gmask2, bmask=bmask, ident=ident2, mq=mq, mk=mk, **f2)
    in_maps2 = []
    for i in range(N_CORES):
        shard = x2[i * NS2:(i + 1) * NS2]          # (256, 512, 16)
        xt = np.ascontiguousarray(shard.transpose(1, 0, 2)).reshape(C, NS2 * NT2)
        in_maps2.append(dict(xt=xt, **common2))
    _CACHE["in_maps2"] = in_maps2
    r2 = run_bass_kernel_spmd(nc2, in_maps2, core_ids=list(range(N_CORES)))
    _CACHE["last_r2"] = [r2.results[i]["yt"] for i in range(N_CORES)]

    out = np.empty((B * H * W, C, NT2), np.float32)
    for i in range(N_CORES):
        yt = r2.results[i]["yt"].reshape(C, NS2, NT2)
        out[i * NS2:(i + 1) * NS2] = yt.transpose(1, 0, 2)
    out = out.reshape(B, H, W, C, NT2).transpose(0, 3, 4, 1, 2)
    return np.ascontiguousarray(out)
